# revision 4
# baseline (speedup 1.0000x reference)
"""GQA attention (B=2,T=2048,D=4096, 32Q/8KV heads, RoPE, causal) on 8 TRN2 cores.

Sharding: core c = (batch b = c//4, head-group g = c%4). Each core computes its
batch's attention for 8 query heads (global 8g..8g+8) + their 2 aligned KV
heads, and applies its slice of wo -> a partial [T, D] output. Host sums the 4
head-group partials per batch. No collectives.

v2 (vs v1 x-stationary): weight-stationary projections in 4 token-block
passes -- lhsT is a wqkv chunk, rhs streams x^T, so Q^T/K^T come out directly
in [head_dim, tok] layout (no PE transposes for Q/K; V needs 4 transposes per
pass). RoPE operates along the partition (head_dim) axis using host-permuted
weight columns (re/im half-split per head) + an SBUF->SBUF half-swapped DMA
copy. Phase B batches exp over strip PAIRS (halves ScalarE per-call overhead)
with odd diagonal strips widened so pairs share a column range; rs/PV
emission is deferred one pair so the PE never waits on ScalarE. Phase C
output-projection groups are interleaved between the last blocks' heads.

All matmuls bf16 (fp8 fails the 2e-2 tolerance; measured in numerics sim).
"""
import numpy as np
import ml_dtypes

import concourse.bass as bass
import concourse.mybir as mybir
from concourse import bacc, tile
from concourse.bass_utils import run_bass_kernel_spmd

bf16 = mybir.dt.bfloat16
e5m2 = mybir.dt.float8e5
f32 = mybir.dt.float32
BF = ml_dtypes.bfloat16
E5 = ml_dtypes.float8_e5m2

B, T, D = 2, 2048, 4096
NQ, NKV, HD = 32, 8, 128
HQ, HKV = 8, 2            # per-core heads
NC = D // 128             # 32 contraction chunks
NB = 4                    # token blocks of 512
NOC = HQ + 2 * HKV        # 12 projection output chunks of 128
SCALE = 1.0 / np.sqrt(HD)
NEG = -30000.0            # fits e5m2; SCALE*NEG ~ -2650 => exp == 0

# oc order within a pass: kv first (unblocks attention earlier), then q.
# global oc index: q heads 0..7, k 8..9, v 10..11
OC_ORDER = [8, 9, 10, 11, 0, 1, 2, 3, 4, 5, 6, 7]


def _build_nc():
    nc = bacc.Bacc(None, target_bir_lowering=False)
    xt_ext = nc.declare_dram_parameter("xt", [NC, 128, T], bf16, isOutput=False)
    wqkv_ext = nc.declare_dram_parameter("wqkv", [NOC, 128, NC, 128], bf16, isOutput=False)
    wo_ext = nc.declare_dram_parameter("wo", [128, HQ, D], bf16, isOutput=False)
    rope_ext = nc.declare_dram_parameter("rope", [2, 128, T], bf16, isOutput=False)
    mask_ext = nc.declare_dram_parameter("mask", [128, 4, 512], e5m2, isOutput=False)
    id_ext = nc.declare_dram_parameter("ident", [128, 128], bf16, isOutput=False)
    out_ext = nc.declare_dram_parameter("out", [T, D], f32, isOutput=True)

    with tile.TileContext(nc) as tc:
        with (
            tc.tile_pool(name="persist", bufs=1) as persist,
            tc.tile_pool(name="xtp", bufs=34) as xtp,
            tc.tile_pool(name="wqp", bufs=2) as wqp,
            tc.tile_pool(name="qtbp", bufs=1) as qtbp,
            tc.tile_pool(name="ropep", bufs=2) as ropep,
            tc.tile_pool(name="qap", bufs=2) as qap,
            tc.tile_pool(name="swp", bufs=2) as swp,
            tc.tile_pool(name="tmpp", bufs=1) as tmpp,
            tc.tile_pool(name="wop", bufs=1) as wop,
            tc.tile_pool(name="ptsp", bufs=3) as ptsp,
            tc.tile_pool(name="recp", bufs=1) as recp,
            tc.tile_pool(name="outp", bufs=3) as outp,
            tc.tile_pool(name="psA", bufs=2, space="PSUM") as psA,
            tc.tile_pool(name="psS", bufs=2, space="PSUM") as psS,
            tc.tile_pool(name="rsB", bufs=1, space="PSUM") as rsB,
            tc.tile_pool(name="otB", bufs=1, space="PSUM") as otB,
        ):
            # ---- persistent small tiles + tables (DMA'd first) ----
            ident = persist.tile([128, 128], bf16, tag="ident")
            nc.sync.dma_start(ident[:], id_ext[:])
            masks = persist.tile([128, 4, 512], e5m2, tag="mask")
            nc.sync.dma_start(masks[:], mask_ext[:])
            ones = persist.tile([128, 128], bf16, tag="ones")
            nc.vector.memset(ones[:], 1.0)

            ktb = [persist.tile([128, HKV, 512], bf16, tag=f"kt{j}", name=f"kt{j}")
                   for j in range(NB)]
            vbb = [persist.tile([128, 4, HKV * 128], bf16, tag=f"vb{j}", name=f"vb{j}")
                   for j in range(NB)]
            aot = persist.tile([128, HQ, T], bf16, tag="aot")
            wo = wop.tile([128, HQ, D], bf16, tag="wo")

            # ---------------- phase B block (one head, one tq-block) ---------
            def b_block(h, b, qtb):
                kvh = h // 4
                nstrip = 4 * (b + 1)
                npair = nstrip // 2
                ot = otB.tile([128, 512], f32, tag="ot")
                rs = rsB.tile([128, 512], f32, tag="rs")

                def emit_sums(pair):
                    t0, lo, pts = pair
                    for j in range(2):
                        t = t0 + j
                        nc.tensor.matmul(
                            rs[:, lo:512], ones[:], pts[:, j, lo:512],
                            start=(t == 0), stop=(t == nstrip - 1))
                        nc.tensor.matmul(
                            ot[:, lo:512],
                            vbb[t // 4][:, t % 4, kvh * 128:(kvh + 1) * 128],
                            pts[:, j, lo:512],
                            start=(t == 0), stop=(t == nstrip - 1))

                prev = None
                for tp in range(npair):
                    t0 = 2 * tp
                    r0 = t0 - 4 * b          # even when >= 0
                    lo = 128 * r0 if r0 > 0 else 0
                    s_ps = psS.tile([128, 2, 512], f32, tag="s")
                    pts = ptsp.tile([128, 2, 512], bf16, tag="pts")
                    for j in range(2):
                        t = t0 + j
                        r = t - 4 * b
                        nc.tensor.matmul(
                            s_ps[:, j, lo:512],
                            ktb[t // 4][:, kvh, (t % 4) * 128:(t % 4 + 1) * 128],
                            qtb[:, h, lo:512],
                            start=True, stop=True)
                        if r == 0:
                            nc.vector.tensor_add(
                                s_ps[:, j, 0:128], s_ps[:, j, 0:128],
                                masks[:, 0, 0:128])
                        elif r > 0:
                            # odd diag strip: widened to the pair's lo; mask
                            # covers [128(r-1), 128(r+1)) (all-NEG + triangle)
                            w0 = 128 * (r - 1) if j == 1 else 128 * r
                            nc.vector.tensor_add(
                                s_ps[:, j, w0:128 * (r + 1)],
                                s_ps[:, j, w0:128 * (r + 1)],
                                masks[:, r, w0:128 * (r + 1)])
                    nc.scalar.activation(
                        pts[:, :, lo:512], s_ps[:, :, lo:512],
                        mybir.ActivationFunctionType.Exp, bias=0.0, scale=SCALE)
                    if prev is not None:
                        emit_sums(prev)
                    prev = (t0, lo, pts)
                emit_sums(prev)
                recip = recp.tile([128, 512], f32, tag="recip")
                nc.vector.reciprocal_approx_fast(out=recip[:], in_=rs[:])
                nc.vector.tensor_mul(
                    aot[:, h, b * 512:(b + 1) * 512], ot[:], recip[:])

            # ---------------- phase C groups -------------------------------
            ost_map = {}

            def c_group(tau, nck):
                o_ps = psA.tile([128, 512], f32, tag="proj")
                for h in range(HQ):
                    nc.tensor.matmul(
                        o_ps[:], aot[:, h, tau * 128:(tau + 1) * 128],
                        wo[:, h, nck * 512:(nck + 1) * 512],
                        start=(h == 0), stop=(h == HQ - 1))
                key = (tau, nck // 2)
                if key not in ost_map:
                    ost_map[key] = outp.tile([128, 1024], f32, tag="ostage",
                                             name=f"ost{tau}_{nck // 2}")
                ost = ost_map[key]
                nc.vector.tensor_copy(ost[:, (nck % 2) * 512:(nck % 2 + 1) * 512],
                                      o_ps[:])
                if nck % 2 == 1:
                    nc.sync.dma_start(
                        out_ext[tau * 128:(tau + 1) * 128,
                                (nck - 1) * 512:(nck + 1) * 512],
                        ost[:])

            c_queue = []

            def emit_c(n):
                for _ in range(min(n, len(c_queue))):
                    tau, nck = c_queue.pop(0)
                    c_group(tau, nck)

            # ---------------- phase A passes + interleaved B/C ---------------
            for p in range(NB):
                xtc = []
                for c in range(NC):
                    xt = xtp.tile([128, 512], bf16, tag="xt", name=f"xt{p}_{c}")
                    nc.sync.dma_start(xt[:], xt_ext[c][:, p * 512:(p + 1) * 512])
                    xtc.append(xt)
                ropecc = ropep.tile([128, 512], bf16, tag="cc")
                ropess = ropep.tile([128, 512], bf16, tag="ss")
                nc.sync.dma_start(ropecc[:], rope_ext[0][:, p * 512:(p + 1) * 512])
                nc.sync.dma_start(ropess[:], rope_ext[1][:, p * 512:(p + 1) * 512])
                qtb = qtbp.tile([128, HQ, 512], bf16, tag="qtb", name=f"qtb{p}")

                for oc in OC_ORDER:
                    wsb = wqp.tile([128, NC, 128], bf16, tag="w", name=f"w{p}_{oc}")
                    nc.gpsimd.dma_start(wsb[:], wqkv_ext[oc])
                    ps = psA.tile([128, 512], f32, tag="proj")
                    for c in range(NC):
                        nc.tensor.matmul(ps[:], wsb[:, c, :], xtc[c][:],
                                         start=(c == 0), stop=(c == NC - 1))
                    if oc < 10:  # q or k head: rope on partitions (half-split)
                        qa = qap.tile([128, 512], bf16, tag="qa")
                        nc.vector.tensor_copy(qa[:], ps[:])
                        sw = swp.tile([128, 512], bf16, tag="sw")
                        nc.sync.dma_start(sw[0:64, :], qa[64:128, :])
                        nc.sync.dma_start(sw[64:128, :], qa[0:64, :])
                        dst = qtb[:, oc, :] if oc < 8 else ktb[p][:, oc - 8, :]
                        tmp = tmpp.tile([128, 512], bf16, tag="tmp")
                        nc.vector.tensor_mul(tmp[:], sw[:], ropess[:])
                        nc.vector.tensor_mul(dst, qa[:], ropecc[:])
                        nc.vector.tensor_add(dst, dst, tmp[:])
                    else:  # v head: transpose [hd,tok] -> [tok,hd]
                        va = qap.tile([128, 512], bf16, tag="qa")
                        nc.vector.tensor_copy(va[:], ps[:])
                        pt = psA.tile([128, 512], bf16, tag="proj", name=f"pt{p}_{oc}")
                        for j in range(4):
                            nc.tensor.transpose(
                                pt[:, j * 128:(j + 1) * 128],
                                va[:, j * 128:(j + 1) * 128], ident[:])
                        kvh = oc - 10
                        nc.vector.tensor_copy(
                            vbb[p][:, :, kvh * 128:(kvh + 1) * 128],
                            pt[:].rearrange("p (j d) -> p j d", j=4))

                if p == NB - 1:
                    nc.gpsimd.dma_start(wo[:], wo_ext[:])

                for h in range(HQ):
                    b_block(h, p, qtb)
                    if p == NB - 1 and h >= 2:
                        emit_c(8)
                for tau in range(4 * p, 4 * p + 4):
                    for nck in range(8):
                        c_queue.append((tau, nck))

            emit_c(len(c_queue))

    nc.compile()
    return nc


def _rope_tables():
    i = np.arange(HD // 2, dtype=np.float64)
    theta = np.power(10000.0, -2.0 * i / HD)
    ang = np.outer(theta, np.arange(T, dtype=np.float64))    # [64, T]
    cc = np.concatenate([np.cos(ang), np.cos(ang)], axis=0)  # [128, T]
    ss = np.concatenate([-np.sin(ang), np.sin(ang)], axis=0)
    return np.ascontiguousarray(np.stack([cc, ss], axis=0)).astype(BF)


def _masks():
    # maskT for S^T strips: partition p = tk within strip, free f = tq within
    # block; strip r (0..3) inside the diagonal region. Valid iff tq >= tk.
    p = np.arange(128)[:, None, None]
    r = np.arange(4)[None, :, None]
    f = np.arange(512)[None, None, :]
    return np.where(f >= 128 * r + p, 0.0, NEG).astype(E5)


def _half_perm():
    # per-head column permutation: d' < 64 -> orig 2d' (re), else 2(d'-64)+1
    d = np.arange(HD)
    return np.where(d < 64, 2 * d, 2 * (d - 64) + 1)


def _prep_core_inputs(x, wq, wk, wv, wo):
    rope = _rope_tables()
    masks = _masks()
    ident = np.eye(128).astype(BF)
    perm = _half_perm()
    in_maps = []
    for c in range(8):
        b, g = c // 4, c % 4
        xb = np.asarray(x[b], dtype=np.float32)          # [T, D]
        xt = np.ascontiguousarray(
            xb.reshape(T, NC, 128).transpose(1, 2, 0)).astype(BF)  # [NC,128,T]
        wq_g = wq[:, g * 8 * HD:(g + 1) * 8 * HD].reshape(D, 8, HD)
        wk_g = wk[:, g * 2 * HD:(g + 1) * 2 * HD].reshape(D, 2, HD)
        wv_g = wv[:, g * 2 * HD:(g + 1) * 2 * HD].reshape(D, 2, HD)
        wq_p = wq_g[:, :, perm]                          # half-split permute
        wk_p = wk_g[:, :, perm]
        occh = [wq_p[:, h, :] for h in range(8)] + \
               [wk_p[:, j, :] for j in range(2)] + \
               [wv_g[:, j, :] for j in range(2)]
        wqkv_t = np.stack([
            np.ascontiguousarray(w.reshape(NC, 128, 128).transpose(1, 0, 2))
            for w in occh], axis=0).astype(BF)           # [NOC, 128, NC, 128]
        wo_g = wo[g * 8 * HD:(g + 1) * 8 * HD, :]        # [1024, D]
        wo_t = np.ascontiguousarray(
            wo_g.reshape(HQ, 128, D).transpose(1, 0, 2)).astype(BF)
        in_maps.append({
            "xt": xt, "wqkv": wqkv_t, "wo": wo_t,
            "rope": rope, "mask": masks, "ident": ident,
        })
    return in_maps


_NC_CACHE = None


def _get_nc():
    global _NC_CACHE
    if _NC_CACHE is None:
        _NC_CACHE = _build_nc()
    return _NC_CACHE


def _run(inputs, trace=False, trace_kwargs=None):
    x = np.asarray(inputs["x"], dtype=np.float32)
    wq = np.asarray(inputs["wq"], dtype=np.float32)
    wk = np.asarray(inputs["wk"], dtype=np.float32)
    wv = np.asarray(inputs["wv"], dtype=np.float32)
    wo = np.asarray(inputs["wo"], dtype=np.float32)
    nc = _get_nc()
    in_maps = _prep_core_inputs(x, wq, wk, wv, wo)
    res = run_bass_kernel_spmd(nc, in_maps, core_ids=list(range(8)),
                               trace=trace, **(trace_kwargs or {}))
    out = np.zeros((B, T, D), dtype=np.float32)
    for c in range(8):
        out[c // 4] += res.results[c]["out"]
    return out, res


def kernel(**inputs):
    out, _ = _run(inputs)
    return out
